# revision 1
# baseline (speedup 1.0000x reference)
"""Trainium2 Bass kernel for nn_ChromaticResonance — v2 redesign.

Math (per batch row, complex wave z, D=512, 7 depths):
  p* = ch @ {C+H1(+I), H2, H3, H5}
  y  = pW + 0.25|p2|^2 (re only) + (1/9)|p3|^2 p3 + 0.04 p5^5 |p5|^-4.8
  t  = tanh(y*s + b);  ch' = fd_d * t;  out += w_d * ch'

Key design points vs the fp32r baseline (4.05ms):
  - State is sigma_d = (w_d*fd_d) . t_d, so out = sum_d sigma_d exactly and
    the fd/w scalings fold into per-depth IMMEDIATES (1/w into the pW/h2/h3
    consumers, w^-0.2 into the exp bias). ONE shared bf16 weight set.
  - All matmuls bf16 (1 cyc/row, FWL weight loads). Two chunks (A/B) are
    interleaved per depth so the PE never waits on a chain tail -> stays at
    2.4GHz (baseline ran ~always HAM-throttled at 1.2).
  - h5 = ((c2*s2)^2) * u with c2 = u^2, s2 = sqrt(0.04 w^-.2 (r^2)^-2.4) from
    a bitcast-log2 Exp — the magnitude correction costs ONE ACT op.
  - Custom DVE ops (sq(a)+-sq(b)) collapse |.|^2 / Re(u^2) to single ops.
  - Chain ops batched 4-wide across m-tiles; engine split ACT/DVE/Pool
    balanced to ~7us per instance-quad.
"""

import numpy as np
import ml_dtypes

import concourse.bass as bass
import concourse.mybir as mybir
import concourse.tile as tile
from concourse import bass_utils
from concourse.bacc import Bacc
import concourse.dve_ops as dve_ops
from concourse.dve_spec import Spec, Src0, Src1, sq

F32 = mybir.dt.float32
BF16 = mybir.dt.bfloat16
I32 = mybir.dt.int32
AF = mybir.ActivationFunctionType
OP = mybir.AluOpType

B, D, DEPTH = 32768, 512, 7
N_CORES = 8
BS = B // N_CORES
NB = 512
KT = D // 128

LN2 = float(np.log(2.0))
SIGMA0 = 0.0430
EXP_SCALE_S2 = float(-1.2 * LN2 * 2.0 ** -23)
_wv = np.exp(-np.linspace(0.0, 2.0, DEPTH))
WV = (_wv / _wv.sum()).astype(np.float64)  # output depth weights (compile-time)

EXP_INT_DIRECT = True  # Exp reads the bitcast int32 tensor directly


def _ebias(dep):
    wprev = 1.0 if dep == 0 else float(WV[dep - 1])
    return (1.2 * LN2 * (127.0 - SIGMA0) + float(np.log(0.2))
            - 0.1 * float(np.log(wprev)))


def _register_custom_ops():
    if "R2_SUM_SQ_ANT" in dve_ops._SUB_OPCODE_FOR_NAME:
        return (dve_ops._R2_SUM_SQ_ANT, dve_ops._CSQ_RE_ANT)
    r2 = dve_ops.DveOp(
        "R2_SUM_SQ_ANT",
        Spec(body=sq(Src0) + sq(Src1),
             reference=lambda in0, in1, s0, s1, imm2: (
                 in0.astype(np.float32) ** 2 + in1.astype(np.float32) ** 2
             ).astype(np.float32)),
        subdim=False,
        uops_sha={"v3": "cd4bd6e1c27efd14", "v4": "121e32d8332f5047"},
    )
    csq = dve_ops.DveOp(
        "CSQ_RE_ANT",
        Spec(body=sq(Src0) - sq(Src1),
             reference=lambda in0, in1, s0, s1, imm2: (
                 in0.astype(np.float32) ** 2 - in1.astype(np.float32) ** 2
             ).astype(np.float32)),
        subdim=False,
        uops_sha={"v3": "fbe824060f113aca", "v4": "765c24b4e00dcf62"},
    )
    for op in (r2, csq):
        dve_ops.OPS.append(op)
        dve_ops.CUSTOM_DVE_SPECS[op.name] = op.spec
        dve_ops._SUB_OPCODE_FOR_NAME[op.name] = (
            dve_ops._CUSTOM_DVE_ROW_BASE + len(dve_ops.OPS) - 1)
    dve_ops._R2_SUM_SQ_ANT = r2
    dve_ops._CSQ_RE_ANT = csq
    return r2, csq


def _dup2(ap):
    """Broadcast a [128, NB] AP to [128, 2, NB] (read twice)."""
    return bass.AP(tensor=ap.tensor, offset=ap.offset,
                   ap=[ap.ap[0], [0, 2], ap.ap[1]])


def build_program(n_chunks=BS // NB, nb=NB):
    assert n_chunks % 2 == 0
    R2OP, CSQOP = _register_custom_ops()
    nc = Bacc()
    bcols = n_chunks * nb
    H = slice(0, nb)
    I = slice(nb, 2 * nb)

    wre = nc.dram_tensor("wre", [D, bcols], BF16, kind="ExternalInput")
    wim = nc.dram_tensor("wim", [D, bcols], BF16, kind="ExternalInput")
    wmat = nc.dram_tensor("wmat", [5, D, D], BF16, kind="ExternalInput")
    consts = nc.dram_tensor("consts", [D, 24], F32, kind="ExternalInput")
    ore = nc.dram_tensor("ore", [D, bcols], BF16, kind="ExternalOutput")
    oim = nc.dram_tensor("oim", [D, bcols], BF16, kind="ExternalOutput")

    # per-depth compile-time constants
    kw_d, k2_d, k3_d, ebias_d = [], [], [], []
    for dep in range(DEPTH):
        wprev = 1.0 if dep == 0 else float(WV[dep - 1])
        kw_d.append(1.0 / wprev)
        k2_d.append(0.25 / wprev ** 2)
        k3_d.append((1.0 / 9.0) / wprev ** 3)
        ebias_d.append(_ebias(dep))

    with tile.TileContext(nc) as tc:
        with (
            tc.tile_pool(name="wpool", bufs=1) as wpool,
            tc.tile_pool(name="spool", bufs=1) as spool,   # sigma states + out
            tc.tile_pool(name="ppool", bufs=1, space="PSUM") as ppool,
            tc.tile_pool(name="cpool", bufs=1) as cpool,   # chain scratch
        ):
            # ---- weights + consts (loaded once) ----
            wt = []
            for mi in range(5):
                w = wpool.tile([128, KT, D], BF16, name=f"wt{mi}", tag=f"wt{mi}")
                for k in range(KT):
                    nc.sync.dma_start(out=w[:, k, :],
                                      in_=wmat[mi, k * 128:(k + 1) * 128, :])
                wt.append(w)
            cons = []
            for m in range(KT):
                c = wpool.tile([128, 24], F32, name=f"cons{m}", tag=f"cons{m}")
                nc.sync.dma_start(out=c, in_=consts[m * 128:(m + 1) * 128, :])
                cons.append(c)

            for cp in range(n_chunks // 2):
                sig = {}
                outs = {}
                for sl in range(2):
                    ci = 2 * cp + sl
                    c0 = ci * nb
                    s0t = spool.tile([128, KT, 2, nb], BF16,
                                     name=f"sg{sl}0", tag=f"sg{sl}0")
                    s1t = spool.tile([128, KT, 2, nb], BF16,
                                     name=f"sg{sl}1", tag=f"sg{sl}1")
                    for k in range(KT):
                        nc.sync.dma_start(
                            out=s0t[:, k, 0, :],
                            in_=wre[k * 128:(k + 1) * 128, c0:c0 + nb])
                        nc.sync.dma_start(
                            out=s0t[:, k, 1, :],
                            in_=wim[k * 128:(k + 1) * 128, c0:c0 + nb])
                    sig[sl] = [s0t, s1t]
                    outs[sl] = spool.tile([128, KT, 2, nb], BF16,
                                          name=f"out{sl}", tag=f"out{sl}", bufs=2)

                for dep in range(DEPTH):
                    w1 = wt[0] if dep == 0 else wt[1]
                    kw, k2, k3 = kw_d[dep], k2_d[dep], k3_d[dep]
                    k3c = float(k3 ** (1.0 / 3.0))
                    out_deferred = []
                    for sl in range(2):
                        scur = sig[sl][dep % 2]
                        snxt = sig[sl][(dep + 1) % 2]
                        out_t = outs[sl]

                        # plane-major scratch: [128, 2(H/I), KT, nb] so every
                        # batched op reads/writes contiguous [128, KT*nb]
                        b3a = cpool.tile([128, 2, KT, nb], BF16,
                                         name="b3a", tag="b3a", bufs=1)
                        b5a = cpool.tile([128, 2, KT, nb], BF16,
                                         name="b5a", tag="b5a", bufs=1)
                        sq2a = cpool.tile([128, 2, KT, nb], BF16,
                                          name="sq2a", tag="sq2a", bufs=1)
                        r3a = cpool.tile([128, KT, nb], BF16,
                                         name="r3a", tag="r3a", bufs=1)
                        r2a = cpool.tile([128, KT, nb], BF16,
                                         name="r2a", tag="r2a", bufs=1)
                        acc = cpool.tile([128, 2, KT, nb], BF16,
                                         name="acc", tag="acc", bufs=2)
                        c2a = cpool.tile([128, 2, KT, nb], BF16,
                                         name="c2a", tag="c2a", bufs=1)
                        c2sa = cpool.tile([128, 2, KT, nb], BF16,
                                          name="c2sa", tag="c2sa", bufs=1)
                        r5 = cpool.tile([128, KT, nb], F32,
                                        name="r5", tag="r5", bufs=1)
                        s2 = cpool.tile([128, KT, nb], BF16,
                                        name="s2", tag="s2", bufs=1)
                        c4a = cpool.tile([128, 2, KT, nb], BF16,
                                         name="c4a", tag="c4a", bufs=1)
                        q2s = cpool.tile([128, KT, nb], BF16,
                                         name="q2s", tag="q2s", bufs=1)
                        q4s = cpool.tile([128, KT, nb], BF16,
                                         name="q4s", tag="q4s", bufs=1)
                        h5 = cpool.tile([128, 2, KT, nb], BF16,
                                        name="h5", tag="h5", bufs=1)

                        for m in range(KT):
                            msl = slice(m * 128, (m + 1) * 128)

                            def mm_group(pt_ap, lw):
                                for k in range(KT):
                                    for j, hs in enumerate((H, I)):
                                        nc.tensor.matmul(
                                            pt_ap[:, hs], lw[:, k, msl],
                                            scur[:, k, j, :],
                                            start=(k == 0), stop=(k == KT - 1))

                            # H3: b3 = k3^(1/3) * p3; r3 later; acc1 (Pool)
                            p3t = ppool.tile([128, 2 * nb], F32,
                                             name="p3", tag="p3")
                            mm_group(p3t[:, :], wt[3])
                            nc.scalar.mul(
                                b3a[:, :, m, :],
                                p3t.rearrange("p (two n) -> p two n", two=2),
                                k3c)

                            # H2: sq2
                            p2t = ppool.tile([128, 2 * nb], F32,
                                             name="p2", tag="p2")
                            mm_group(p2t[:, :], wt[2])
                            nc.scalar.activation(
                                sq2a[:, :, m, :],
                                p2t.rearrange("p (two n) -> p two n", two=2),
                                AF.Square, scale=float(k2 ** 0.5))

                            # H5: b5 copy
                            p5t = ppool.tile([128, 2 * nb], F32,
                                             name="p5", tag="p5")
                            mm_group(p5t[:, :], wt[4])
                            nc.scalar.copy(
                                b5a[:, :, m, :],
                                p5t.rearrange("p (two n) -> p two n", two=2))

                            # W1: acc2 = kw*pW + acc1  (after acc1 below)
                            pWt = ppool.tile([128, 2 * nb], F32,
                                             name="pW", tag="pW")
                            mm_group(pWt[:, :], w1)
                            # per-instance r3 + acc1 (Pool) + acc2 (DVE 2x)
                            nc.vector._custom_dve(
                                R2OP, out=r3a[:, m, :],
                                in0=b3a[:, 0, m, :], in1=b3a[:, 1, m, :])
                            nc.vector.tensor_tensor(
                                acc[:, :, m, :], b3a[:, :, m, :],
                                _dup2(r3a[:, m, :]), op=OP.mult)
                            bw = cpool.tile([128, 2, nb], BF16,
                                            name="bw", tag="bw", bufs=3)
                            nc.scalar.mul(
                                bw[:, :, :],
                                pWt.rearrange("p (two n) -> p two n", two=2),
                                kw)
                            nc.vector.tensor_tensor(
                                acc[:, :, m, :], acc[:, :, m, :],
                                bw[:, :, :], op=OP.add)

                        # ---- batched chain (contiguous planes) ----
                        b5H, b5I = b5a[:, 0, :, :], b5a[:, 1, :, :]
                        nc.gpsimd.tensor_tensor(
                            r2a[:, :, :], sq2a[:, 0, :, :], sq2a[:, 1, :, :],
                            op=OP.add)
                        c2r, c2ih = c2a[:, 0, :, :], c2a[:, 1, :, :]
                        nc.vector._custom_dve(CSQOP, out=c2r,
                                              in0=b5H, in1=b5I)
                        nc.gpsimd.tensor_tensor(c2ih, b5H, b5I, op=OP.mult)
                        nc.vector._custom_dve(R2OP, out=r5[:, :, :],
                                              in0=b5H, in1=b5I)
                        nc.scalar.activation(
                            s2[:, :, :], r5[:, :, :].bitcast(I32), AF.Exp,
                            scale=EXP_SCALE_S2,
                            bias=cons[0][:, 9 + dep:10 + dep])
                        s2d = cpool.tile([128, KT, nb], BF16,
                                         name="s2d", tag="s2d", bufs=1)
                        nc.scalar.activation(
                            s2d[:, :, :], r5[:, :, :].bitcast(I32), AF.Exp,
                            scale=EXP_SCALE_S2,
                            bias=cons[0][:, 16 + dep:17 + dep])
                        c2sr, c2si = c2sa[:, 0, :, :], c2sa[:, 1, :, :]
                        nc.vector.tensor_tensor(c2sr, c2r, s2[:, :, :],
                                                op=OP.mult)
                        nc.vector.tensor_tensor(c2si, c2ih, s2d[:, :, :],
                                                op=OP.mult)
                        A, c4n = c4a[:, 0, :, :], c4a[:, 1, :, :]
                        nc.vector._custom_dve(CSQOP, out=A,
                                              in0=c2sr, in1=c2si)
                        # c4n = -2*c2sr*c2si = -(t5*c4i): h5 combines are 2x TTs
                        nc.vector.scalar_tensor_tensor(
                            c4n, c2sr, -2.0, c2si, op0=OP.mult, op1=OP.mult)
                        h5H, h5I = h5[:, 0, :, :], h5[:, 1, :, :]
                        nc.vector.tensor_tensor(h5H, A, b5H, op=OP.mult)
                        nc.vector.tensor_tensor(q2s[:, :, :], c4n, b5I,
                                                op=OP.mult)
                        nc.vector.tensor_tensor(h5H, h5H, q2s[:, :, :],
                                                op=OP.add)
                        nc.vector.tensor_tensor(h5I, A, b5I, op=OP.mult)
                        nc.vector.tensor_tensor(q4s[:, :, :], c4n, b5H,
                                                op=OP.mult)
                        nc.vector.tensor_tensor(h5I, h5I, q4s[:, :, :],
                                                op=OP.subtract)
                        # acc: += r2 (pre-scaled by k2 in Square), += h5
                        accH = acc[:, 0, :, :]
                        nc.vector.tensor_tensor(accH, accH, r2a[:, :, :],
                                                op=OP.add)
                        # per-m: acc += h5, tanh, sigma (DVE TS 4x) — each
                        # next-depth k-tile can start on its own sigma
                        for m in range(KT):
                            nc.vector.tensor_tensor(
                                acc[:, :, m, :], acc[:, :, m, :],
                                h5[:, :, m, :], op=OP.add)
                            nc.scalar.activation(
                                snxt[:, m, :, :], acc[:, :, m, :], AF.Tanh,
                                scale=cons[m][:, 7:8], bias=cons[m][:, 8:9])
                            nc.scalar.mul(snxt[:, m, :, :],
                                          snxt[:, m, :, :],
                                          cons[m][:, dep:dep + 1])
                            out_deferred.append((out_t, snxt, m, dep))

                    for (ot, sn, mm_, dd) in out_deferred:
                        if dd == 0:
                            nc.gpsimd.tensor_copy(ot[:, mm_, :, :],
                                                  sn[:, mm_, :, :])
                        else:
                            nc.gpsimd.tensor_tensor(
                                ot[:, mm_, :, :], ot[:, mm_, :, :],
                                sn[:, mm_, :, :], op=OP.add)

                for sl in range(2):
                    ci = 2 * cp + sl
                    c0 = ci * nb
                    for m in range(KT):
                        nc.sync.dma_start(
                            out=ore[m * 128:(m + 1) * 128, c0:c0 + nb],
                            in_=outs[sl][:, m, 0, :])
                        nc.sync.dma_start(
                            out=oim[m * 128:(m + 1) * 128, c0:c0 + nb],
                            in_=outs[sl][:, m, 1, :])
    nc.finalize()
    return nc


def host_prep(coupling_matrix, harmonic_1, harmonic_2, harmonic_3, harmonic_5,
              mixing_scale, mixing_bias):
    damping = (0.1 / (1.0 + np.exp(np.linspace(0.0, 3.0, D)))).astype(np.float64)
    fd = np.stack([np.exp(-damping * dd) for dd in range(DEPTH)])  # [7, D]
    wf = (WV[:, None] * fd).astype(np.float32)                     # [7, D]
    w1_0 = (coupling_matrix + harmonic_1).astype(np.float32)
    w1_r = w1_0 + np.eye(D, dtype=np.float32)
    wmat = np.ascontiguousarray(
        np.stack([w1_0, w1_r, harmonic_2, harmonic_3, harmonic_5])
    ).astype(ml_dtypes.bfloat16)
    consts = np.zeros((D, 24), np.float32)
    consts[:, 0:DEPTH] = wf.T
    consts[:, 7] = mixing_scale.astype(np.float32)
    consts[:, 8] = mixing_bias.astype(np.float32)
    for dep in range(DEPTH):
        consts[:, 9 + dep] = _ebias(dep)
        consts[:, 16 + dep] = _ebias(dep) + float(np.log(2.0))
    return wmat, consts


_NC_CACHE = {}


def _get_nc(n_chunks, nb):
    key = (n_chunks, nb)
    if key not in _NC_CACHE:
        _NC_CACHE[key] = build_program(n_chunks, nb)
    return _NC_CACHE[key]


def kernel(wave_real, wave_imag, coupling_matrix, harmonic_1, harmonic_2,
           harmonic_3, harmonic_5, mixing_scale, mixing_bias):
    wmat, consts = host_prep(coupling_matrix, harmonic_1, harmonic_2,
                             harmonic_3, harmonic_5, mixing_scale, mixing_bias)
    wreT = np.asarray(wave_real, np.float32).T.astype(ml_dtypes.bfloat16)
    wimT = np.asarray(wave_imag, np.float32).T.astype(ml_dtypes.bfloat16)

    nc = _get_nc(BS // NB, NB)
    in_maps = []
    for c in range(N_CORES):
        sl = slice(c * BS, (c + 1) * BS)
        in_maps.append({
            "wre": np.ascontiguousarray(wreT[:, sl]),
            "wim": np.ascontiguousarray(wimT[:, sl]),
            "wmat": wmat,
            "consts": consts,
        })
    res = bass_utils.run_bass_kernel_spmd(nc, in_maps, core_ids=list(range(N_CORES)))
    out = np.empty((2, B, D), np.float32)
    for c in range(N_CORES):
        sl = slice(c * BS, (c + 1) * BS)
        out[0, sl, :] = res.results[c]["ore"].astype(np.float32).T
        out[1, sl, :] = res.results[c]["oim"].astype(np.float32).T
    return out



# revision 2
# speedup vs baseline: 1.3686x; 1.3686x over previous
"""Trainium2 Bass kernel for nn_ChromaticResonance — v3 redesign.

Math (per batch row, complex wave z, D=512, 7 depths):
  p* = ch @ {C+H1(+I), H2, H3, H5}
  y  = pW + 0.25|p2|^2 (re only) + (1/9)|p3|^2 p3 + (1/25) p5^5 |p5|^-4.8
  t  = tanh(y*s + b);  ch' = fd_d * t;  out += w_d * ch'

v3 changes vs v2 (2.707ms):
  The v2 trace showed DVE at 90% busy (2.43ms active) while PE sat at 69%
  with 30% HAM-cold time — the elementwise chain, not the matmuls, was the
  bottleneck. v3 slashes DVE work with two fused custom DVE ops:
   - CUBE3_ANT:   h3 = b3*(|b3|^2) per component in ONE op (replaces
     R2 + dup2-mult).
   - QUINTIC5_ANT: Re(w^5) = x*((x^2-A*y^2)*(x^2-B*y^2)) with A,B roots of
     t^2-10t+5 — the whole order-5 harmonic combine in 2 ops (replaces the
     c2/c2s/c4/6xTT complex-multiply chain, ~11 DVE ops + 1 Exp + 1 GPS op).
     The |p5|^-4.8 magnitude correction pre-scales p5: w = p5 * s15 with
     s15 = (1/25)^.2 wprev^-.08 |p5|^-.96 from ONE bitcast-log2 Exp; then
     h5 = w^5 exactly (quintic in fp32 inside the DVE pipe).
  Out-accumulation moved to one whole-tile GpSimd op; sigma scaling moved
  to DVE tensor_scalar (4x mode). Per-slot engine budget: PE 27.3us,
  DVE ~22us, ACT ~23us, GPS ~6us -> PE-bound.
"""

import numpy as np
import ml_dtypes

import concourse.bass as bass
import concourse.mybir as mybir
import concourse.tile as tile
from concourse import bass_utils
from concourse.bacc import Bacc
import concourse.dve_ops as dve_ops
from concourse.dve_spec import Spec, Src0, Src1, C0, C1, sq, lower, _has_src1
from concourse.dve_uop import DveOpSpec

F32 = mybir.dt.float32
BF16 = mybir.dt.bfloat16
I32 = mybir.dt.int32
AF = mybir.ActivationFunctionType
OP = mybir.AluOpType

B, D, DEPTH = 32768, 512, 7
N_CORES = 8
BS = B // N_CORES
NB = 512
KT = D // 128

LN2 = float(np.log(2.0))
SIGMA0 = 0.0430
EXP_SCALE15 = float(-0.48 * LN2 * 2.0 ** -23)
_wv = np.exp(-np.linspace(0.0, 2.0, DEPTH))
WV = (_wv / _wv.sum()).astype(np.float64)  # output depth weights (compile-time)

QA = float(5.0 + 2.0 * np.sqrt(5.0))  # x^4-10x^2y^2+5y^4 = (x^2-QA y^2)(x^2-QB y^2)
QB = float(5.0 - 2.0 * np.sqrt(5.0))


def _bias15(dep):
    wprev = 1.0 if dep == 0 else float(WV[dep - 1])
    return (0.48 * LN2 * (127.0 - SIGMA0) + 0.2 * float(np.log(1.0 / 25.0))
            - 0.08 * float(np.log(wprev)))


def _sha_of(spec, name):
    shas = {}
    for ver in ("v3", "v4"):
        uops = lower(spec, ver=ver)
        shas[ver] = DveOpSpec(name=name, opcode=None, uops=uops,
                              rd1_en=_has_src1(spec)).sha(ver)
    return shas


def _register_custom_ops():
    if "R2_SUM_SQ_ANT" in dve_ops._SUB_OPCODE_FOR_NAME:
        return (dve_ops._R2_SUM_SQ_ANT, dve_ops._CUBE3_ANT, dve_ops._QUINTIC5_ANT)
    r2 = dve_ops.DveOp(
        "R2_SUM_SQ_ANT",
        Spec(body=sq(Src0) + sq(Src1),
             reference=lambda in0, in1, s0, s1, imm2: (
                 in0.astype(np.float32) ** 2 + in1.astype(np.float32) ** 2
             ).astype(np.float32)),
        subdim=False,
        uops_sha={"v3": "cd4bd6e1c27efd14", "v4": "121e32d8332f5047"},
    )
    cube3_spec = Spec(
        body=Src0 * (sq(Src0) + sq(Src1)),
        reference=lambda in0, in1, s0, s1, imm2: (
            in0.astype(np.float32)
            * (in0.astype(np.float32) ** 2 + in1.astype(np.float32) ** 2)
        ).astype(np.float32))
    cube3 = dve_ops.DveOp("CUBE3_ANT", cube3_spec, subdim=False,
                          uops_sha=_sha_of(cube3_spec, "CUBE3_ANT"))

    def _quintic_ref(in0, in1, s0, s1, imm2):
        x = in0.astype(np.float32)
        y = in1.astype(np.float32)
        return (x * ((x * x - s0 * y * y) * (x * x - s1 * y * y))).astype(np.float32)

    _sx, _sy = sq(Src0), sq(Src1)
    quintic_spec = Spec(body=Src0 * ((_sx - _sy * C0) * (_sx - _sy * C1)),
                        reference=_quintic_ref)
    quintic = dve_ops.DveOp("QUINTIC5_ANT", quintic_spec, subdim=False,
                            uops_sha=_sha_of(quintic_spec, "QUINTIC5_ANT"))
    for op in (r2, cube3, quintic):
        dve_ops.OPS.append(op)
        dve_ops.CUSTOM_DVE_SPECS[op.name] = op.spec
        dve_ops._SUB_OPCODE_FOR_NAME[op.name] = (
            dve_ops._CUSTOM_DVE_ROW_BASE + len(dve_ops.OPS) - 1)
    dve_ops._R2_SUM_SQ_ANT = r2
    dve_ops._CUBE3_ANT = cube3
    dve_ops._QUINTIC5_ANT = quintic
    return r2, cube3, quintic


def build_program(n_chunks=BS // NB, nb=NB):
    assert n_chunks % 2 == 0
    R2OP, CUBE3, QUINT = _register_custom_ops()
    nc = Bacc()
    bcols = n_chunks * nb

    wre = nc.dram_tensor("wre", [D, bcols], BF16, kind="ExternalInput")
    wim = nc.dram_tensor("wim", [D, bcols], BF16, kind="ExternalInput")
    wmat = nc.dram_tensor("wmat", [5, D, D], BF16, kind="ExternalInput")
    consts = nc.dram_tensor("consts", [D, 24], F32, kind="ExternalInput")
    ore = nc.dram_tensor("ore", [D, bcols], BF16, kind="ExternalOutput")
    oim = nc.dram_tensor("oim", [D, bcols], BF16, kind="ExternalOutput")

    # per-depth compile-time constants
    kw_d, k2_d, k3c_d = [], [], []
    for dep in range(DEPTH):
        wprev = 1.0 if dep == 0 else float(WV[dep - 1])
        kw_d.append(1.0 / wprev)
        k2_d.append(0.25 / wprev ** 2)
        k3c_d.append(float(((1.0 / 9.0) / wprev ** 3) ** (1.0 / 3.0)))

    with tile.TileContext(nc) as tc:
        with (
            tc.tile_pool(name="wpool", bufs=1) as wpool,
            tc.tile_pool(name="spool", bufs=1) as spool,   # sigma states + out
            tc.tile_pool(name="ppool", bufs=1, space="PSUM") as ppool,
            tc.tile_pool(name="cpool", bufs=1) as cpool,   # chain scratch
        ):
            # ---- weights + consts (loaded once) ----
            wt = []
            for mi in range(5):
                w = wpool.tile([128, KT, D], BF16, name=f"wt{mi}", tag=f"wt{mi}")
                for k in range(KT):
                    nc.sync.dma_start(out=w[:, k, :],
                                      in_=wmat[mi, k * 128:(k + 1) * 128, :])
                wt.append(w)
            cons = []
            for m in range(KT):
                c = wpool.tile([128, 24], F32, name=f"cons{m}", tag=f"cons{m}")
                nc.sync.dma_start(out=c, in_=consts[m * 128:(m + 1) * 128, :])
                cons.append(c)

            for cp in range(n_chunks // 2):
                sig = {}
                outs = {}
                for sl in range(2):
                    ci = 2 * cp + sl
                    c0 = ci * nb
                    s0t = spool.tile([128, KT, 2, nb], BF16,
                                     name=f"sg{sl}0", tag=f"sg{sl}0")
                    s1t = spool.tile([128, KT, 2, nb], BF16,
                                     name=f"sg{sl}1", tag=f"sg{sl}1")
                    for k in range(KT):
                        nc.sync.dma_start(
                            out=s0t[:, k, 0, :],
                            in_=wre[k * 128:(k + 1) * 128, c0:c0 + nb])
                        nc.sync.dma_start(
                            out=s0t[:, k, 1, :],
                            in_=wim[k * 128:(k + 1) * 128, c0:c0 + nb])
                    sig[sl] = [s0t, s1t]
                    outs[sl] = spool.tile([128, KT, 2, nb], BF16,
                                          name=f"out{sl}", tag=f"out{sl}", bufs=1)

                for dep in range(DEPTH):
                    w1 = wt[0] if dep == 0 else wt[1]
                    kw, k2, k3c = kw_d[dep], k2_d[dep], k3c_d[dep]
                    for sl in range(2):
                        scur = sig[sl][dep % 2]
                        snxt = sig[sl][(dep + 1) % 2]
                        out_t = outs[sl]

                        # chain scratch, double-buffered so the two
                        # interleaved chunks never share live tiles
                        b3 = cpool.tile([128, 2, KT, nb], BF16,
                                        name="b3", tag="b3q5", bufs=2)
                        sq2 = cpool.tile([128, 2, KT, nb], BF16,
                                         name="sq2", tag="sq2", bufs=2)
                        b5 = cpool.tile([128, 2, KT, nb], BF16,
                                        name="b5", tag="b5", bufs=2)
                        bw = cpool.tile([128, 2, KT, nb], BF16,
                                        name="bw", tag="bw", bufs=2)
                        acc = cpool.tile([128, 2, KT, nb], BF16,
                                         name="acc", tag="acc", bufs=2)
                        r5 = cpool.tile([128, KT, nb], F32,
                                        name="r5", tag="r5", bufs=2)
                        s15 = cpool.tile([128, KT, nb], BF16,
                                         name="s15", tag="s15", bufs=2)
                        q5 = cpool.tile([128, 2, KT, nb], BF16,
                                        name="q5", tag="b3q5", bufs=2)

                        for m in range(KT):
                            msl = slice(m * 128, (m + 1) * 128)
                            H = slice(0, nb)
                            I = slice(nb, 2 * nb)

                            def mm_group(pt_ap, lw):
                                for k in range(KT):
                                    for j, hs in enumerate((H, I)):
                                        nc.tensor.matmul(
                                            pt_ap[:, hs], lw[:, k, msl],
                                            scur[:, k, j, :],
                                            start=(k == 0), stop=(k == KT - 1))

                            # H5 first: its chain tail is the longest
                            p5t = ppool.tile([128, 2 * nb], F32,
                                             name="p5", tag="p5")
                            mm_group(p5t[:, :], wt[4])
                            nc.scalar.copy(
                                b5[:, :, m, :],
                                p5t.rearrange("p (two n) -> p two n", two=2))

                            p3t = ppool.tile([128, 2 * nb], F32,
                                             name="p3", tag="p3")
                            mm_group(p3t[:, :], wt[3])
                            nc.scalar.mul(
                                b3[:, :, m, :],
                                p3t.rearrange("p (two n) -> p two n", two=2),
                                k3c)

                            p2t = ppool.tile([128, 2 * nb], F32,
                                             name="p2", tag="p2")
                            mm_group(p2t[:, :], wt[2])
                            nc.scalar.activation(
                                sq2[:, :, m, :],
                                p2t.rearrange("p (two n) -> p two n", two=2),
                                AF.Square, scale=float(k2 ** 0.5))

                            pWt = ppool.tile([128, 2 * nb], F32,
                                             name="pW", tag="pW")
                            mm_group(pWt[:, :], w1)
                            nc.scalar.mul(
                                bw[:, :, m, :],
                                pWt.rearrange("p (two n) -> p two n", two=2),
                                kw)

                        # ---- batched chain (contiguous planes) ----
                        b3H, b3I = b3[:, 0, :, :], b3[:, 1, :, :]
                        accH, accI = acc[:, 0, :, :], acc[:, 1, :, :]
                        b5H, b5I = b5[:, 0, :, :], b5[:, 1, :, :]

                        # h5 magnitude prescale first: r5 -> s15 -> w (ACT Exp
                        # overlaps the DVE h3/h2 work below)
                        nc.vector._custom_dve(R2OP, out=r5[:, :, :],
                                              in0=b5H, in1=b5I)
                        nc.scalar.activation(
                            s15[:, :, :], r5[:, :, :].bitcast(I32), AF.Exp,
                            scale=EXP_SCALE15,
                            bias=cons[0][:, 9 + dep:10 + dep])

                        # h3 = b3*|b3|^2 per component, writes acc
                        nc.vector._custom_dve(CUBE3, out=accH, in0=b3H, in1=b3I)
                        nc.vector._custom_dve(CUBE3, out=accI, in0=b3I, in1=b3H)
                        # + linear part (both planes in one op)
                        nc.vector.tensor_tensor(acc[:, :, :, :], acc[:, :, :, :],
                                                bw[:, :, :, :], op=OP.add)
                        # + h2 (real only)
                        nc.vector.tensor_tensor(accH, accH, sq2[:, 0, :, :],
                                                op=OP.add)
                        nc.vector.tensor_tensor(accH, accH, sq2[:, 1, :, :],
                                                op=OP.add)
                        # w = p5 * s15 (in place), h5 = w^5 via quintic
                        nc.vector.tensor_tensor(b5H, b5H, s15[:, :, :],
                                                op=OP.mult)
                        nc.vector.tensor_tensor(b5I, b5I, s15[:, :, :],
                                                op=OP.mult)
                        nc.vector._custom_dve(QUINT, out=q5[:, 0, :, :],
                                              in0=b5H, in1=b5I, s0=QA, s1=QB)
                        nc.vector._custom_dve(QUINT, out=q5[:, 1, :, :],
                                              in0=b5I, in1=b5H, s0=QA, s1=QB)
                        nc.vector.tensor_tensor(acc[:, :, :, :], acc[:, :, :, :],
                                                q5[:, :, :, :], op=OP.add)

                        # tanh + sigma per m so next-depth k-tiles start early
                        for m in range(KT):
                            nc.scalar.activation(
                                snxt[:, m, :, :], acc[:, :, m, :], AF.Tanh,
                                scale=cons[m][:, 7:8], bias=cons[m][:, 8:9])
                            nc.vector.tensor_scalar_mul(
                                snxt[:, m, :, :], snxt[:, m, :, :],
                                cons[m][:, dep:dep + 1])

                        # out accumulation off the critical path on GpSimd
                        if dep == 0:
                            nc.gpsimd.tensor_copy(out_t[:, :, :, :],
                                                  snxt[:, :, :, :])
                        else:
                            nc.gpsimd.tensor_tensor(
                                out_t[:, :, :, :], out_t[:, :, :, :],
                                snxt[:, :, :, :], op=OP.add)

                for sl in range(2):
                    ci = 2 * cp + sl
                    c0 = ci * nb
                    for m in range(KT):
                        nc.sync.dma_start(
                            out=ore[m * 128:(m + 1) * 128, c0:c0 + nb],
                            in_=outs[sl][:, m, 0, :])
                        nc.sync.dma_start(
                            out=oim[m * 128:(m + 1) * 128, c0:c0 + nb],
                            in_=outs[sl][:, m, 1, :])
    nc.finalize()
    return nc


def host_prep(coupling_matrix, harmonic_1, harmonic_2, harmonic_3, harmonic_5,
              mixing_scale, mixing_bias):
    damping = (0.1 / (1.0 + np.exp(np.linspace(0.0, 3.0, D)))).astype(np.float64)
    fd = np.stack([np.exp(-damping * dd) for dd in range(DEPTH)])  # [7, D]
    wf = (WV[:, None] * fd).astype(np.float32)                     # [7, D]
    w1_0 = (coupling_matrix + harmonic_1).astype(np.float32)
    w1_r = w1_0 + np.eye(D, dtype=np.float32)
    wmat = np.ascontiguousarray(
        np.stack([w1_0, w1_r, harmonic_2, harmonic_3, harmonic_5])
    ).astype(ml_dtypes.bfloat16)
    consts = np.zeros((D, 24), np.float32)
    consts[:, 0:DEPTH] = wf.T
    consts[:, 7] = mixing_scale.astype(np.float32)
    consts[:, 8] = mixing_bias.astype(np.float32)
    for dep in range(DEPTH):
        consts[:, 9 + dep] = _bias15(dep)
    return wmat, consts


_NC_CACHE = {}


def _get_nc(n_chunks, nb):
    key = (n_chunks, nb)
    if key not in _NC_CACHE:
        _NC_CACHE[key] = build_program(n_chunks, nb)
    return _NC_CACHE[key]


def kernel(wave_real, wave_imag, coupling_matrix, harmonic_1, harmonic_2,
           harmonic_3, harmonic_5, mixing_scale, mixing_bias):
    wmat, consts = host_prep(coupling_matrix, harmonic_1, harmonic_2,
                             harmonic_3, harmonic_5, mixing_scale, mixing_bias)
    wreT = np.asarray(wave_real, np.float32).T.astype(ml_dtypes.bfloat16)
    wimT = np.asarray(wave_imag, np.float32).T.astype(ml_dtypes.bfloat16)

    nc = _get_nc(BS // NB, NB)
    in_maps = []
    for c in range(N_CORES):
        sl = slice(c * BS, (c + 1) * BS)
        in_maps.append({
            "wre": np.ascontiguousarray(wreT[:, sl]),
            "wim": np.ascontiguousarray(wimT[:, sl]),
            "wmat": wmat,
            "consts": consts,
        })
    res = bass_utils.run_bass_kernel_spmd(nc, in_maps, core_ids=list(range(N_CORES)))
    out = np.empty((2, B, D), np.float32)
    for c in range(N_CORES):
        sl = slice(c * BS, (c + 1) * BS)
        out[0, sl, :] = res.results[c]["ore"].astype(np.float32).T
        out[1, sl, :] = res.results[c]["oim"].astype(np.float32).T
    return out


# revision 3
# speedup vs baseline: 1.6649x; 1.2165x over previous
"""Trainium2 Bass kernel for nn_ChromaticResonance — v3 redesign.

Math (per batch row, complex wave z, D=512, 7 depths):
  p* = ch @ {C+H1(+I), H2, H3, H5}
  y  = pW + 0.25|p2|^2 (re only) + (1/9)|p3|^2 p3 + (1/25) p5^5 |p5|^-4.8
  t  = tanh(y*s + b);  ch' = fd_d * t;  out += w_d * ch'

v3 changes vs v2 (2.707ms):
  The v2 trace showed DVE at 90% busy (2.43ms active) while PE sat at 69%
  with 30% HAM-cold time — the elementwise chain, not the matmuls, was the
  bottleneck. v3 slashes DVE work with two fused custom DVE ops:
   - CUBE3_ANT:   h3 = b3*(|b3|^2) per component in ONE op (replaces
     R2 + dup2-mult).
   - QUINTIC5_ANT: Re(w^5) = x*((x^2-A*y^2)*(x^2-B*y^2)) with A,B roots of
     t^2-10t+5 — the whole order-5 harmonic combine in 2 ops (replaces the
     c2/c2s/c4/6xTT complex-multiply chain, ~11 DVE ops + 1 Exp + 1 GPS op).
     The |p5|^-4.8 magnitude correction pre-scales p5: w = p5 * s15 with
     s15 = (1/25)^.2 wprev^-.08 |p5|^-.96 from ONE bitcast-log2 Exp; then
     h5 = w^5 exactly (quintic in fp32 inside the DVE pipe).
  Out-accumulation moved to one whole-tile GpSimd op; sigma scaling moved
  to DVE tensor_scalar (4x mode). Per-slot engine budget: PE 27.3us,
  DVE ~22us, ACT ~23us, GPS ~6us -> PE-bound.
"""

import numpy as np
import ml_dtypes

import concourse.bass as bass
import concourse.mybir as mybir
import concourse.tile as tile
from concourse import bass_utils
from concourse.bacc import Bacc
import concourse.dve_ops as dve_ops
from concourse.dve_spec import Spec, Src0, Src1, C0, C1, sq, lower, _has_src1
from concourse.dve_uop import DveOpSpec

F32 = mybir.dt.float32
BF16 = mybir.dt.bfloat16
I32 = mybir.dt.int32
AF = mybir.ActivationFunctionType
OP = mybir.AluOpType

B, D, DEPTH = 32768, 512, 7
N_CORES = 8
BS = B // N_CORES
NB = 512
KT = D // 128

LN2 = float(np.log(2.0))
SIGMA0 = 0.0430
EXP_SCALE15 = float(-0.48 * LN2 * 2.0 ** -23)
_wv = np.exp(-np.linspace(0.0, 2.0, DEPTH))
WV = (_wv / _wv.sum()).astype(np.float64)  # output depth weights (compile-time)

QA = float(5.0 + 2.0 * np.sqrt(5.0))  # x^4-10x^2y^2+5y^4 = (x^2-QA y^2)(x^2-QB y^2)
QB = float(5.0 - 2.0 * np.sqrt(5.0))


def _bias15(dep):
    wprev = 1.0 if dep == 0 else float(WV[dep - 1])
    return (0.48 * LN2 * (127.0 - SIGMA0) + 0.2 * float(np.log(1.0 / 25.0))
            - 0.08 * float(np.log(wprev)))


def _sha_of(spec, name):
    shas = {}
    for ver in ("v3", "v4"):
        uops = lower(spec, ver=ver)
        shas[ver] = DveOpSpec(name=name, opcode=None, uops=uops,
                              rd1_en=_has_src1(spec)).sha(ver)
    return shas


def _register_custom_ops():
    if "R2_SUM_SQ_ANT" in dve_ops._SUB_OPCODE_FOR_NAME:
        return (dve_ops._R2_SUM_SQ_ANT, dve_ops._CUBE3_ANT, dve_ops._QUINTIC5_ANT)
    r2 = dve_ops.DveOp(
        "R2_SUM_SQ_ANT",
        Spec(body=sq(Src0) + sq(Src1),
             reference=lambda in0, in1, s0, s1, imm2: (
                 in0.astype(np.float32) ** 2 + in1.astype(np.float32) ** 2
             ).astype(np.float32)),
        subdim=False,
        uops_sha={"v3": "cd4bd6e1c27efd14", "v4": "121e32d8332f5047"},
    )
    cube3_spec = Spec(
        body=Src0 * (sq(Src0) + sq(Src1)),
        reference=lambda in0, in1, s0, s1, imm2: (
            in0.astype(np.float32)
            * (in0.astype(np.float32) ** 2 + in1.astype(np.float32) ** 2)
        ).astype(np.float32))
    cube3 = dve_ops.DveOp("CUBE3_ANT", cube3_spec, subdim=False,
                          uops_sha=_sha_of(cube3_spec, "CUBE3_ANT"))

    def _quintic_ref(in0, in1, s0, s1, imm2):
        x = in0.astype(np.float32)
        y = in1.astype(np.float32)
        return (x * ((x * x - s0 * y * y) * (x * x - s1 * y * y))).astype(np.float32)

    _sx, _sy = sq(Src0), sq(Src1)
    quintic_spec = Spec(body=Src0 * ((_sx - _sy * C0) * (_sx - _sy * C1)),
                        reference=_quintic_ref)
    quintic = dve_ops.DveOp("QUINTIC5_ANT", quintic_spec, subdim=False,
                            uops_sha=_sha_of(quintic_spec, "QUINTIC5_ANT"))
    for op in (r2, cube3, quintic):
        dve_ops.OPS.append(op)
        dve_ops.CUSTOM_DVE_SPECS[op.name] = op.spec
        dve_ops._SUB_OPCODE_FOR_NAME[op.name] = (
            dve_ops._CUSTOM_DVE_ROW_BASE + len(dve_ops.OPS) - 1)
    dve_ops._R2_SUM_SQ_ANT = r2
    dve_ops._CUBE3_ANT = cube3
    dve_ops._QUINTIC5_ANT = quintic
    return r2, cube3, quintic


def build_program(n_chunks=BS // NB, nb=NB):
    assert n_chunks % 2 == 0
    R2OP, CUBE3, QUINT = _register_custom_ops()
    nc = Bacc()
    bcols = n_chunks * nb

    wre = nc.dram_tensor("wre", [D, bcols], BF16, kind="ExternalInput")
    wim = nc.dram_tensor("wim", [D, bcols], BF16, kind="ExternalInput")
    wmat = nc.dram_tensor("wmat", [5, D, D], BF16, kind="ExternalInput")
    consts = nc.dram_tensor("consts", [D, 24], F32, kind="ExternalInput")
    ore = nc.dram_tensor("ore", [D, bcols], BF16, kind="ExternalOutput")
    oim = nc.dram_tensor("oim", [D, bcols], BF16, kind="ExternalOutput")

    # per-depth compile-time constants
    kw_d, k2_d, k3c_d = [], [], []
    for dep in range(DEPTH):
        wprev = 1.0 if dep == 0 else float(WV[dep - 1])
        kw_d.append(1.0 / wprev)
        k2_d.append(0.25 / wprev ** 2)
        k3c_d.append(float(((1.0 / 9.0) / wprev ** 3) ** (1.0 / 3.0)))

    with tile.TileContext(nc) as tc:
        with (
            tc.tile_pool(name="wpool", bufs=1) as wpool,
            tc.tile_pool(name="spool", bufs=1) as spool,   # sigma states + out
            tc.tile_pool(name="ppool", bufs=1, space="PSUM") as ppool,
            tc.tile_pool(name="cpool", bufs=1) as cpool,   # chain scratch
        ):
            # ---- weights + consts (loaded once) ----
            wt = []
            for mi in range(5):
                w = wpool.tile([128, KT, D], BF16, name=f"wt{mi}", tag=f"wt{mi}")
                for k in range(KT):
                    nc.sync.dma_start(out=w[:, k, :],
                                      in_=wmat[mi, k * 128:(k + 1) * 128, :])
                wt.append(w)
            cons = []
            for m in range(KT):
                c = wpool.tile([128, 24], F32, name=f"cons{m}", tag=f"cons{m}")
                nc.sync.dma_start(out=c, in_=consts[m * 128:(m + 1) * 128, :])
                cons.append(c)

            for cp in range(n_chunks // 2):
                sig = {}
                outs = {}
                for sl in range(2):
                    ci = 2 * cp + sl
                    c0 = ci * nb
                    s0t = spool.tile([128, KT, 2, nb], BF16,
                                     name=f"sg{sl}0", tag=f"sg{sl}0")
                    s1t = spool.tile([128, KT, 2, nb], BF16,
                                     name=f"sg{sl}1", tag=f"sg{sl}1")
                    for k in range(KT):
                        nc.sync.dma_start(
                            out=s0t[:, k, 0, :],
                            in_=wre[k * 128:(k + 1) * 128, c0:c0 + nb])
                        nc.sync.dma_start(
                            out=s0t[:, k, 1, :],
                            in_=wim[k * 128:(k + 1) * 128, c0:c0 + nb])
                    sig[sl] = [s0t, s1t]
                    outs[sl] = spool.tile([128, KT, 2, nb], BF16,
                                          name=f"out{sl}", tag=f"out{sl}", bufs=1)

                for dep in range(DEPTH):
                    w1 = wt[0] if dep == 0 else wt[1]
                    kw, k2, k3c = kw_d[dep], k2_d[dep], k3c_d[dep]
                    for sl in range(2):
                        scur = sig[sl][dep % 2]
                        snxt = sig[sl][(dep + 1) % 2]
                        out_t = outs[sl]

                        # chain scratch, double-buffered so the two
                        # interleaved chunks never share live tiles
                        b3 = cpool.tile([128, 2, KT, nb], BF16,
                                        name="b3", tag="b3q5", bufs=2)
                        sq2 = cpool.tile([128, 2, KT, nb], BF16,
                                         name="sq2", tag="sq2", bufs=2)
                        b5 = cpool.tile([128, 2, KT, nb], BF16,
                                        name="b5", tag="b5", bufs=2)
                        bw = cpool.tile([128, 2, KT, nb], BF16,
                                        name="bw", tag="bw", bufs=2)
                        acc = cpool.tile([128, 2, KT, nb], BF16,
                                         name="acc", tag="acc", bufs=2)
                        r5 = cpool.tile([128, KT, nb], F32,
                                        name="r5", tag="r5", bufs=2)
                        s15 = cpool.tile([128, KT, nb], BF16,
                                         name="s15", tag="s15", bufs=2)
                        q5 = cpool.tile([128, 2, KT, nb], BF16,
                                        name="q5", tag="b3q5", bufs=2)

                        for m in range(KT):
                            msl = slice(m * 128, (m + 1) * 128)
                            H = slice(0, nb)
                            I = slice(nb, 2 * nb)

                            def mm_group(pt_ap, lw):
                                for k in range(KT):
                                    for j, hs in enumerate((H, I)):
                                        nc.tensor.matmul(
                                            pt_ap[:, hs], lw[:, k, msl],
                                            scur[:, k, j, :],
                                            start=(k == 0), stop=(k == KT - 1))

                            # H5 first: its chain tail is the longest
                            p5t = ppool.tile([128, 2 * nb], F32,
                                             name="p5", tag="p5")
                            mm_group(p5t[:, :], wt[4])
                            nc.scalar.copy(
                                b5[:, :, m, :],
                                p5t.rearrange("p (two n) -> p two n", two=2))

                            p3t = ppool.tile([128, 2 * nb], F32,
                                             name="p3", tag="p3")
                            mm_group(p3t[:, :], wt[3])
                            nc.scalar.mul(
                                b3[:, :, m, :],
                                p3t.rearrange("p (two n) -> p two n", two=2),
                                k3c)

                            p2t = ppool.tile([128, 2 * nb], F32,
                                             name="p2", tag="p2")
                            mm_group(p2t[:, :], wt[2])
                            nc.scalar.activation(
                                sq2[:, :, m, :],
                                p2t.rearrange("p (two n) -> p two n", two=2),
                                AF.Square, scale=float(k2 ** 0.5))

                            pWt = ppool.tile([128, 2 * nb], F32,
                                             name="pW", tag="pW")
                            mm_group(pWt[:, :], w1)
                            nc.scalar.mul(
                                bw[:, :, m, :],
                                pWt.rearrange("p (two n) -> p two n", two=2),
                                kw)

                        # ---- batched chain (contiguous planes) ----
                        b3H, b3I = b3[:, 0, :, :], b3[:, 1, :, :]
                        accH, accI = acc[:, 0, :, :], acc[:, 1, :, :]
                        b5H, b5I = b5[:, 0, :, :], b5[:, 1, :, :]

                        # h5 magnitude prescale first: r5 -> s15 -> w (ACT Exp
                        # overlaps the DVE h3/h2 work below)
                        nc.vector._custom_dve(R2OP, out=r5[:, :, :],
                                              in0=b5H, in1=b5I)
                        nc.scalar.activation(
                            s15[:, :, :], r5[:, :, :].bitcast(I32), AF.Exp,
                            scale=EXP_SCALE15,
                            bias=cons[0][:, 9 + dep:10 + dep])

                        # h3 = b3*|b3|^2 per component, writes acc
                        nc.vector._custom_dve(CUBE3, out=accH, in0=b3H, in1=b3I)
                        nc.vector._custom_dve(CUBE3, out=accI, in0=b3I, in1=b3H)
                        # + linear part (both planes in one op)
                        nc.vector.tensor_tensor(acc[:, :, :, :], acc[:, :, :, :],
                                                bw[:, :, :, :], op=OP.add)
                        # + h2 (real only)
                        nc.vector.tensor_tensor(accH, accH, sq2[:, 0, :, :],
                                                op=OP.add)
                        nc.vector.tensor_tensor(accH, accH, sq2[:, 1, :, :],
                                                op=OP.add)
                        # w = p5 * s15 (in place), h5 = w^5 via quintic
                        nc.vector.tensor_tensor(b5H, b5H, s15[:, :, :],
                                                op=OP.mult)
                        nc.vector.tensor_tensor(b5I, b5I, s15[:, :, :],
                                                op=OP.mult)
                        nc.vector._custom_dve(QUINT, out=q5[:, 0, :, :],
                                              in0=b5H, in1=b5I, s0=QA, s1=QB)
                        nc.vector._custom_dve(QUINT, out=q5[:, 1, :, :],
                                              in0=b5I, in1=b5H, s0=QA, s1=QB)
                        nc.vector.tensor_tensor(acc[:, :, :, :], acc[:, :, :, :],
                                                q5[:, :, :, :], op=OP.add)

                        # tanh + sigma per m so next-depth k-tiles start early
                        for m in range(KT):
                            nc.scalar.activation(
                                snxt[:, m, :, :], acc[:, :, m, :], AF.Tanh,
                                scale=cons[m][:, 7:8], bias=cons[m][:, 8:9])
                            nc.vector.tensor_scalar_mul(
                                snxt[:, m, :, :], snxt[:, m, :, :],
                                cons[m][:, dep:dep + 1])

                        # out accumulation on DVE: any GpSimd op would grab the
                        # shared SBUF port pair and block concurrent DVE ops
                        # (trace showed the chain's first custom stalled for
                        # the full GpSimd op duration, every slot)
                        if dep == 0:
                            nc.vector.tensor_copy(out_t[:, :, :, :],
                                                  snxt[:, :, :, :])
                        else:
                            nc.vector.tensor_tensor(
                                out_t[:, :, :, :], out_t[:, :, :, :],
                                snxt[:, :, :, :], op=OP.add)

                for sl in range(2):
                    ci = 2 * cp + sl
                    c0 = ci * nb
                    for m in range(KT):
                        nc.sync.dma_start(
                            out=ore[m * 128:(m + 1) * 128, c0:c0 + nb],
                            in_=outs[sl][:, m, 0, :])
                        nc.sync.dma_start(
                            out=oim[m * 128:(m + 1) * 128, c0:c0 + nb],
                            in_=outs[sl][:, m, 1, :])
    nc.finalize()
    return nc


def host_prep(coupling_matrix, harmonic_1, harmonic_2, harmonic_3, harmonic_5,
              mixing_scale, mixing_bias):
    damping = (0.1 / (1.0 + np.exp(np.linspace(0.0, 3.0, D)))).astype(np.float64)
    fd = np.stack([np.exp(-damping * dd) for dd in range(DEPTH)])  # [7, D]
    wf = (WV[:, None] * fd).astype(np.float32)                     # [7, D]
    w1_0 = (coupling_matrix + harmonic_1).astype(np.float32)
    w1_r = w1_0 + np.eye(D, dtype=np.float32)
    wmat = np.ascontiguousarray(
        np.stack([w1_0, w1_r, harmonic_2, harmonic_3, harmonic_5])
    ).astype(ml_dtypes.bfloat16)
    consts = np.zeros((D, 24), np.float32)
    consts[:, 0:DEPTH] = wf.T
    consts[:, 7] = mixing_scale.astype(np.float32)
    consts[:, 8] = mixing_bias.astype(np.float32)
    for dep in range(DEPTH):
        consts[:, 9 + dep] = _bias15(dep)
    return wmat, consts


_NC_CACHE = {}


def _get_nc(n_chunks, nb):
    key = (n_chunks, nb)
    if key not in _NC_CACHE:
        _NC_CACHE[key] = build_program(n_chunks, nb)
    return _NC_CACHE[key]


def kernel(wave_real, wave_imag, coupling_matrix, harmonic_1, harmonic_2,
           harmonic_3, harmonic_5, mixing_scale, mixing_bias):
    wmat, consts = host_prep(coupling_matrix, harmonic_1, harmonic_2,
                             harmonic_3, harmonic_5, mixing_scale, mixing_bias)
    wreT = np.asarray(wave_real, np.float32).T.astype(ml_dtypes.bfloat16)
    wimT = np.asarray(wave_imag, np.float32).T.astype(ml_dtypes.bfloat16)

    nc = _get_nc(BS // NB, NB)
    in_maps = []
    for c in range(N_CORES):
        sl = slice(c * BS, (c + 1) * BS)
        in_maps.append({
            "wre": np.ascontiguousarray(wreT[:, sl]),
            "wim": np.ascontiguousarray(wimT[:, sl]),
            "wmat": wmat,
            "consts": consts,
        })
    res = bass_utils.run_bass_kernel_spmd(nc, in_maps, core_ids=list(range(N_CORES)))
    out = np.empty((2, B, D), np.float32)
    for c in range(N_CORES):
        sl = slice(c * BS, (c + 1) * BS)
        out[0, sl, :] = res.results[c]["ore"].astype(np.float32).T
        out[1, sl, :] = res.results[c]["oim"].astype(np.float32).T
    return out
